# revision 38
# baseline (speedup 1.0000x reference)
"""Trainium2 Bass kernel for the two-tower GCN (nn_GCN2).

Distribution: nodes partitioned by destination range across 8 cores
(graph parallel). Edge lists are preprocessed on host (index manipulation
only): assigned to the core owning their dst node, sorted by dst tile,
and padded so every core runs the identical program. Gather indices are
stored relative to the table midpoint (signed int16 sign-extends in the
gather ucode), so one index stream covers all 50176 table rows.

All floating-point math runs on device across 3 SPMD launches:
  A: xW   = x @ [W1|W3]  (f16 matmul, fp8 output table)
  B: h^T  = relu(spmm(A, xW) + b), o = hW2      (per dst tile, fp8 gather)
  C: oT   = spmm(A, hW2); gated fusion; log_softmax  ([dst, class] layout)

The irregular segment-sum runs as selector-matrix matmuls on the tensor
engine: for each chunk of 128 edges, sel[e, d] = val_e * (dstloc_e == d)
is built on the vector engine from a constant iota, and the per-edge
gathered feature rows (via dma_gather) contract against it in PSUM.
"""
from contextlib import ExitStack

import numpy as np

import concourse.bass as bass
import concourse.tile as tile
from concourse import bacc, mybir
from concourse.bass_utils import run_bass_kernel_spmd
from concourse.masks import make_identity

P = 128
NCORES = 8
N = 50000
NFEAT = 512
NHID = 128
NCLASS = 40
NLOC = N // NCORES            # 6250 real nodes per core
NTILE = (NLOC + P - 1) // P   # 49 dst tiles per core
NLOCP = NTILE * P             # 6272 padded rows per core
NPAD = NCORES * NLOCP         # 50176 padded table rows
MID = NPAD // 2               # gather base row (signed idx spans the table)
G = 8                         # chunks per gather batch
R = G * P                     # 1024 indices per dma_gather (ucode max)

f8 = mybir.dt.float8e4
f16 = mybir.dt.float16
f32 = mybir.dt.float32
i16 = mybir.dt.int16
i32 = mybir.dt.int32
ACT = mybir.ActivationFunctionType
ALU = mybir.AluOpType

NP_F8 = mybir.dt.np(f8)
XW_F8 = False                 # launch-B gather table in fp8 (else f16)


def _cdiv(a, b):
    return (a + b - 1) // b


# ---------------------------------------------------------------- host prep

class TowerPlan:
    """Edge preprocessing for one tower (one graph).

    chunk_cnt  : [NTILE] chunks per dst tile (same for all cores)
    idx        : [NCORES, nb, 128, R//16] int16 mid-relative gather indices
    dl, vl     : [NCORES, 128, nchunks] dstloc (f32) / edge values (f32)
    nchunks, nb, last_R
    """

    def __init__(self, edge_index, edge_vals):
        src = np.asarray(edge_index[0]).astype(np.int64)
        dst = np.asarray(edge_index[1]).astype(np.int64)
        vals = np.asarray(edge_vals).astype(np.float32)

        core = dst // NLOC
        ldst = dst - core * NLOC

        counts = np.bincount(core, minlength=NCORES)              # [NCORES]
        self.nchunks = max(int(_cdiv(counts.max(), P)), 1)
        self.nb = _cdiv(self.nchunks, G)
        self.last_R = (self.nchunks - (self.nb - 1) * G) * P

        # slot position of each edge: rank within the core's dst-sorted
        # stream — chunks are NOT tile-aligned (no per-tile padding)
        order = np.lexsort((ldst, core))
        so_core = core[order]
        so_ldst, so_src, so_val = ldst[order], src[order], vals[order]
        gstart = np.r_[0, np.flatnonzero(np.diff(so_core)) + 1]
        glen = np.diff(np.r_[gstart, len(so_core)])
        rank = np.arange(len(so_core)) - np.repeat(gstart, glen)

        nslot = self.nchunks * P
        nb = self.nb
        srcrel = np.zeros((NCORES, nslot), np.int32)   # pad idx 0 (row MID)
        dla = np.zeros((NCORES, nslot), np.float32)
        vla = np.zeros((NCORES, nslot), np.float32)    # pad val 0
        lda = np.full((NCORES, nslot), -1, np.int64)   # ldst, -1 = pad
        flat = so_core * nslot + rank
        srcrel.reshape(-1)[flat] = (so_src - MID).astype(np.int32)
        lda.reshape(-1)[flat] = so_ldst
        vla.reshape(-1)[flat] = so_val

        # uniform chunk -> tile-window map: W[c] = min over cores of the
        # first real edge's tile; chunks span <= 2 consecutive tiles across
        # all cores (inter-core quantile jitter << 128 dsts)
        ldc = lda.reshape(NCORES, self.nchunks, P)
        real = ldc >= 0
        ftile = np.where(real, ldc // P, NTILE).min(axis=2)       # [NCORES, nc]
        ltile = np.where(real, ldc // P, -1).max(axis=2)
        W = ftile.min(axis=0)
        mx = ltile.max(axis=0)
        # pad-only chunks (tail): pin to the last window
        none = mx < 0
        W[none] = NTILE - 1
        mx[none] = NTILE - 1
        W = np.minimum(W, mx)
        assert (mx - W <= 1).all(), "chunk spans >2 tiles"
        self.win = W                                   # [nchunks]
        self.wide = mx - W                             # 0 = single tile
        dla.reshape(NCORES, self.nchunks, P)[:] = np.where(
            real, ldc - (W * P)[None, :, None], 0.0)

        # per chunk: (tile, window-offset, is_first, is_last) events
        first_c = np.full(NTILE, -1)
        last_c = np.zeros(NTILE, np.int64)
        for c in range(self.nchunks):
            for t in range(W[c], mx[c] + 1):
                if first_c[t] < 0:
                    first_c[t] = c
                last_c[t] = c
        self.cevents = [[] for _ in range(self.nchunks)]
        for c in range(self.nchunks):
            for t in range(W[c], mx[c] + 1):
                self.cevents[c].append(
                    (t, t - W[c], first_c[t] == c, last_c[t] == c))

        # The gather ucode stops at the last non-negative index (trailing
        # negatives read as padding). Swap a non-negative slot into each
        # batch's final position; slot order within a chunk is free.
        for cc in range(NCORES):
            for b in range(nb):
                last = min((b + 1) * R, nslot) - 1
                if srcrel[cc, last] >= 0:
                    continue
                c0 = (last // P) * P
                j = c0 + int(np.argmax(srcrel[cc, c0:last + 1] >= 0))
                assert srcrel[cc, j] >= 0, "all-negative chunk tail"
                for arr in (srcrel, dla, vla):
                    arr[cc, [j, last]] = arr[cc, [last, j]]

        # wrapped idx [NCORES, nb, 128, R//16]; pad idx 0 beyond nslot
        w = np.zeros((NCORES, nb * R), np.int32)
        w[:, :nslot] = srcrel
        w = w.reshape(NCORES, nb, R)
        jj = np.arange(R)
        wr = np.zeros((NCORES, nb, 16, R // 16), np.int16)
        wr[:, :, jj % 16, jj // 16] = w.astype(np.int16)
        self.idx = np.tile(wr, (1, 1, 8, 1))           # [NCORES, nb, 128, R//16]

        assert (first_c >= 0).all()

        def colmaj(a):
            out = a.reshape(NCORES, self.nchunks, P).astype(np.float32)
            return np.ascontiguousarray(out.transpose(0, 2, 1))

        self.dl = colmaj(dla)
        self.vl = colmaj(vla)


# ---------------------------------------------------------------- kernels

def _dma_gather_small(gp, out_ap, in_ap, idxs_ap, num_idxs, num_idxs_reg,
                      elem_size, elem_step, queue_num=0):
    """dma_gather for elem sizes below 256B (non-transpose DRAM path only).

    bass.dma_gather asserts elem_size_bytes % 256 == 0, but that alignment is
    only required by the transpose ucode; the plain path only needs the row
    stride in 256B units. Mirrors bass.py's lowering minus that assert.
    """
    from concourse import ap_utils
    from concourse._compat import exact_div
    assert idxs_ap.dtype == i16
    assert in_ap.dtype == out_ap.dtype
    assert in_ap.space == bass.MemorySpace.DRAM
    assert ap_utils.ap_is_contiguous(out_ap.ap[1:])
    assert ap_utils.ap_is_contiguous(idxs_ap.ap[1:])
    assert in_ap.ap[-1][1] == out_ap.ap[-1][1] == elem_size
    assert out_ap.ap[0][1] * out_ap.ap[1][1] == _cdiv(num_idxs, P) * P
    assert in_ap.ap[0][0] == elem_step
    stride_bytes = elem_step * mybir.dt.size(in_ap.dtype)
    stride_bytes_256 = exact_div(stride_bytes, 256)
    _in_ap = gp.lower_ap_dma(in_ap, for_custom_bir_dma=True)
    _idxs_ap = gp.lower_ap(idxs_ap)
    _out_ap = gp.lower_ap(out_ap)
    return gp.add_instruction(mybir.InstDMAGatherAnt(
        name=gp.bass.get_next_instruction_name(),
        ins=[*_in_ap, _idxs_ap, gp.lower_val_access(gp.to_reg(num_idxs_reg))],
        outs=[_out_ap],
        transpose=False, num_idxs=num_idxs, elem_size=elem_size,
        stride_bytes_256=stride_bytes_256, gen_mode=0, single_packet=True,
        queue_num=queue_num, sbuf_tokens_per_rank=0, sbuf_free_dim_per_rank=0,
        sbuf_free_dim_pad_per_rank=0, sbuf_byte_offset=0,
    ))


def _iota_const(nc, ctx, tc):
    pool = ctx.enter_context(tc.tile_pool(name="iotac", bufs=1))
    it32 = pool.tile([P, 2 * P], i32)
    nc.gpsimd.iota(it32[:], pattern=[[1, 2 * P]], base=0, channel_multiplier=0)
    it16 = pool.tile([P, 2 * P], f16)
    nc.vector.tensor_copy(it16[:], it32[:])
    return it16


def build_A(nc):
    xT = nc.dram_tensor("xT", [NFEAT, NLOCP], f32, kind="ExternalInput").ap()
    w13 = nc.dram_tensor("w13", [NFEAT, 2 * NHID], f32, kind="ExternalInput").ap()
    odt = f8 if XW_F8 else f16
    out = nc.dram_tensor("out", [NLOCP, 2 * NHID], odt, kind="ExternalOutput").ap()
    KCH = NFEAT // P  # 4

    TB = 7                    # dst tiles per column block
    NBLK = NTILE // TB        # 7 blocks
    COLB = TB * P             # 896

    with tile.TileContext(nc) as tc, ExitStack() as ctx:
        big = ctx.enter_context(tc.tile_pool(name="big", bufs=1))
        xf_pool = ctx.enter_context(tc.tile_pool(name="xf", bufs=5))
        psum = ctx.enter_context(tc.tile_pool(name="ps", bufs=4, space="PSUM"))

        w_t = []
        for k in range(KCH):
            t = big.tile([P, 2 * NHID], f32, tag=f"w{k}")
            nc.sync.dma_start(t[:], w13[k * P:(k + 1) * P, :])
            tb = big.tile([P, 2 * NHID], f16, tag=f"wb{k}")
            nc.vector.tensor_copy(tb[:], t[:])
            w_t.append(tb)
        ob = big.tile([P, NTILE, 2 * NHID], odt, tag="ob")

        orr = out.rearrange("(t p) f -> p t f", p=P)
        for blk in range(NBLK):
            xt_t = []
            for k in range(KCH):
                t = xf_pool.tile([P, COLB], f32, tag=f"xt{k}")
                nc.sync.dma_start(
                    t[:], xT[k * P:(k + 1) * P, blk * COLB:(blk + 1) * COLB]
                )
                tb = xf_pool.tile([P, COLB], f16, tag=f"xb{k}")
                nc.scalar.copy(tb[:], t[:])
                xt_t.append(tb)
            for rr in range(TB):
                r = blk * TB + rr
                ps = psum.tile([P, 2 * NHID], f32, tag="ps")
                for k in range(KCH):
                    nc.tensor.matmul(
                        ps[:],
                        lhsT=xt_t[k][:, rr * P:(rr + 1) * P],
                        rhs=w_t[k][:],
                        start=(k == 0), stop=(k == KCH - 1),
                    )
                nc.vector.tensor_copy(ob[:, r, :], ps[:])
            nc.sync.dma_start(orr[:, blk * TB:(blk + 1) * TB, :],
                              ob[:, blk * TB:(blk + 1) * TB, :])
    nc.compile()
    return nc


def _emit_spmm_batches(nc, state, tw, b):
    """Lazily emit gather + selector build for batch b of tower tw."""
    key = (tw, b)
    if key in state["batches"]:
        return state["batches"][key]
    plan, pools = state["plans"][tw], state["pools"]
    iota = state["iota"]
    nbq = state["q"]
    state["q"] += 1

    elem = state["elem"]
    nidx = plan.last_R if b == plan.nb - 1 else R
    gcnt = nidx // P
    msgs = pools["msgs"].tile([P, G, elem], state["mdt"], tag="msgs")
    _dma_gather_small(
        nc.gpsimd, msgs[:, 0:gcnt, :], state["tabs"][tw],
        state["idx"][tw][:, b, 0:nidx // 16],
        num_idxs=nidx, num_idxs_reg=nidx,
        elem_size=elem, elem_step=state["tab_step"],
        queue_num=nbq % 2,
    )
    sel = pools["sel"].tile([P, G, 2 * P], f16, tag="sel")
    dl = state["dl"][tw]
    vl = state["vl"][tw]
    for g in range(gcnt):
        c = b * G + g
        wd = (1 + plan.wide[c]) * P
        nc.vector.tensor_scalar(
            out=sel[:, g, 0:wd], in0=iota[:, 0:wd],
            scalar1=dl[:, c:c + 1], scalar2=vl[:, c:c + 1],
            op0=ALU.is_equal, op1=ALU.mult,
        )
    state["batches"][key] = (msgs, sel)
    return msgs, sel


def _load_edge_inputs(nc, ctx, tc, plans):
    """Declare + load idx/dl/vl tensors for both towers.

    Loads are split so the first gathers and selector builds only wait on
    a small head slice, not the whole multi-MB edge stream.
    """
    idx_t, dl_t, vl_t = {}, {}, {}
    pool = ctx.enter_context(tc.tile_pool(name="edges", bufs=1))
    HB = 10                    # head: 10 gather batches worth of everything
    SB = 512                   # tail slice: one DMA per array
    tails = []                 # (position_key, dst_ap, src_ap)
    for tw in (0, 1):
        plan = plans[tw]
        nb, nch = plan.nb, plan.nchunks
        d_idx = nc.dram_tensor(
            f"idx{tw}", [nb, P, R // 16], i16, kind="ExternalInput"
        ).ap()
        t_idx = pool.tile([P, nb, R // 16], i16, tag=f"idx{tw}")
        d_idx_r = d_idx.rearrange("b p w -> p b w")
        nc.sync.dma_start(t_idx[:, 0:HB, :], d_idx_r[:, 0:HB, :])
        idx_t[tw] = t_idx
        d_dl = nc.dram_tensor(
            f"dl{tw}", [P, nch], f32, kind="ExternalInput"
        ).ap()
        t_dl = pool.tile([P, nch], f32, tag=f"dl{tw}")
        nc.sync.dma_start(t_dl[:, 0:HB * G], d_dl[:, 0:HB * G])
        dl_t[tw] = t_dl
        d_vl = nc.dram_tensor(
            f"vl{tw}", [P, nch], f32, kind="ExternalInput"
        ).ap()
        t_vl = pool.tile([P, nch], f32, tag=f"vl{tw}")
        nc.sync.dma_start(t_vl[:, 0:HB * G], d_vl[:, 0:HB * G])
        vl_t[tw] = t_vl
        for b0 in range(HB, nb, SB):
            b1 = min(b0 + SB, nb)
            c0, c1 = b0 * G, min(b1 * G, nch)
            tails.append((b0, t_idx[:, b0:b1, :], d_idx_r[:, b0:b1, :]))
            tails.append((b0, t_dl[:, c0:c1], d_dl[:, c0:c1]))
            tails.append((b0, t_vl[:, c0:c1], d_vl[:, c0:c1]))
    for _, dst, src in sorted(tails, key=lambda x: x[0]):
        nc.sync.dma_start(dst, src)
    return idx_t, dl_t, vl_t


def build_B(nc, plans):
    tdt = f8 if XW_F8 else f16
    xw = nc.dram_tensor("xw", [NPAD, 2 * NHID], tdt, kind="ExternalInput").ap()
    w24 = nc.dram_tensor("w24", [NHID, 2 * NCLASS], f16, kind="ExternalInput").ap()
    b13 = nc.dram_tensor("b13", [NHID, 2], f32, kind="ExternalInput").ap()
    out = nc.dram_tensor("out", [NLOCP, 2 * NCLASS], f16, kind="ExternalOutput").ap()

    with tile.TileContext(nc) as tc, ExitStack() as ctx:
        iota = _iota_const(nc, ctx, tc)
        idx_t, dl_t, vl_t = _load_edge_inputs(nc, ctx, tc, plans)
        consts = ctx.enter_context(tc.tile_pool(name="consts", bufs=1))
        w24_t = consts.tile([NHID, 2 * NCLASS], f16)
        nc.sync.dma_start(w24_t[:], w24[:])
        b13_t = consts.tile([NHID, 2], f32)
        nc.sync.dma_start(b13_t[:], b13[:])
        ob = consts.tile([P, NTILE, 2 * NCLASS], f16, tag="ob")

        pools = {
            "msgs": ctx.enter_context(tc.tile_pool(name="msgs", bufs=12)),
            "sel": ctx.enter_context(tc.tile_pool(name="sel", bufs=12)),
        }
        psum = ctx.enter_context(tc.tile_pool(name="ps", bufs=3, space="PSUM"))
        psum_o = ctx.enter_context(tc.tile_pool(name="pso", bufs=2, space="PSUM"))
        hpool = ctx.enter_context(tc.tile_pool(name="h", bufs=3))

        state = {
            "plans": plans, "pools": pools, "iota": iota, "q": 0,
            "batches": {}, "idx": idx_t, "dl": dl_t, "vl": vl_t,
            "tabs": [xw[MID:, 0:NHID], xw[MID:, NHID:2 * NHID]],
            "tab_step": 2 * NHID,
            "elem": NHID,
            "mdt": tdt,
        }

        orr = out.rearrange("(t p) f -> p t f", p=P)
        for tw in (0, 1):
            plan = plans[tw]
            ps_tiles = {}
            for c in range(plan.nchunks):
                b, g = divmod(c, G)
                msgs, sel = _emit_spmm_batches(nc, state, tw, b)
                for t, w, is_f, is_l in plan.cevents[c]:
                    if is_f:
                        ps_tiles[t] = psum.tile([NHID, P], f32, tag="psh",
                                                name=f"psh{tw}_{t}")
                    nc.tensor.matmul(
                        ps_tiles[t][:], lhsT=msgs[:, g, :],
                        rhs=sel[:, g, w * P:(w + 1) * P],
                        start=is_f, stop=is_l,
                    )
                    if not is_l:
                        continue
                    ps_h = ps_tiles.pop(t)
                    hT = hpool.tile([NHID, P], f16, tag="hT")
                    nc.scalar.activation(
                        out=hT[:], in_=ps_h[:], func=ACT.Relu,
                        bias=b13_t[:, tw:tw + 1], scale=1.0,
                    )
                    ps_o = psum_o.tile([P, NCLASS], f32, tag="pso")
                    nc.tensor.matmul(
                        ps_o[:], lhsT=hT[:],
                        rhs=w24_t[:, tw * NCLASS:(tw + 1) * NCLASS],
                        start=True, stop=True,
                    )
                    nc.scalar.copy(ob[:, t, tw * NCLASS:(tw + 1) * NCLASS],
                                   ps_o[:])
                    # stream output per tile group once both towers are done
                    if tw == 1 and (t % 7 == 6 or t == NTILE - 1):
                        lo = t - t % 7
                        nc.sync.dma_start(orr[:, lo:t + 1, :],
                                          ob[:, lo:t + 1, :])
    nc.compile()
    return nc


def build_C(nc, plans):
    # hw2 table: f16, 256B row stride; tower tw's 40 cols start at tw*64
    hw2 = nc.dram_tensor("hw2", [NPAD, 128], f16, kind="ExternalInput").ap()
    # wl padded to 128 rows: rows 0:40 = Wl[0:40], rows 64:104 = Wl[40:80]
    wl = nc.dram_tensor("wl", [P, NCLASS], f16, kind="ExternalInput").ap()
    b24 = nc.dram_tensor("b24", [1, 2 * NCLASS], f16, kind="ExternalInput").ap()
    nbl = nc.dram_tensor("nbl", [NCLASS, 1], f32, kind="ExternalInput").ap()
    out = nc.dram_tensor("out", [NLOCP, NCLASS], f32, kind="ExternalOutput").ap()

    with tile.TileContext(nc) as tc, ExitStack() as ctx:
        iota = _iota_const(nc, ctx, tc)
        idx_t, dl_t, vl_t = _load_edge_inputs(nc, ctx, tc, plans)
        consts = ctx.enter_context(tc.tile_pool(name="consts", bufs=1))
        wl_t = consts.tile([P, NCLASS], f16)
        nc.sync.dma_start(wl_t[:], wl[:])
        b24_t = consts.tile([1, 2 * NCLASS], f16)
        nc.sync.dma_start(b24_t[:], b24[:])
        nbl_t = consts.tile([NCLASS, 1], f32)
        nc.sync.dma_start(nbl_t[:], nbl[:])
        ones_t = consts.tile([1, P], f16, tag="ones")
        nc.vector.memset(ones_t[:], 1.0)
        ident = consts.tile([P, P], f16, tag="ident")
        make_identity(nc, ident[:])
        # fused per-tower outputs [dst, (tw0 cols 0:40 | tw1 cols 64:104)];
        # zeroed once so the cat transpose emits clean zero fill rows
        o_cat = consts.tile([P, NTILE, P], f16, tag="o_cat")
        nc.vector.memset(o_cat[:], 0.0)
        # logits + softmax stats, ln'd once at the end
        l_all = consts.tile([P, NTILE, NCLASS], f16, tag="l_all")
        ob = consts.tile([P, NTILE, NCLASS], f32, tag="ob")
        negmax_all = consts.tile([P, NTILE], f32, tag="negmax")
        esum_all = consts.tile([P, NTILE], f32, tag="esum")
        lse_all = consts.tile([P, NTILE], f32, tag="lse")

        pools = {
            "msgs": ctx.enter_context(tc.tile_pool(name="msgs", bufs=12)),
            "sel": ctx.enter_context(tc.tile_pool(name="sel", bufs=12)),
        }
        work = ctx.enter_context(tc.tile_pool(name="work", bufs=4))

        state = {
            "plans": plans, "pools": pools, "iota": iota, "q": 0,
            "batches": {}, "idx": idx_t, "dl": dl_t, "vl": vl_t,
            "tabs": [hw2[MID:, 0:64], hw2[MID:, 64:128]],
            "tab_step": 128,
            "elem": 64,
            "mdt": f16,
        }

        # ---- per tile: accumulate both towers' spmm in PSUM, then fuse
        acc_pool = ctx.enter_context(tc.tile_pool(name="acc", bufs=3,
                                                  space="PSUM"))
        eps = ctx.enter_context(tc.tile_pool(name="eps", bufs=2, space="PSUM"))
        orr = out.rearrange("(t p) f -> p t f", p=P)
        def fuse_tile(t):
            # gated fusion + log_softmax stats for tile t. Only Exp runs on
            # the scalar engine here -- any other activation function would
            # trigger a 1.3us table reload per switch.
            o1 = o_cat[:, t, 0:NCLASS]
            o2 = o_cat[:, t, 64:64 + NCLASS]
            # catT [128, P]: one transpose; fill rows come out zero
            ps_cat = eps.tile([P, P], f16, tag="cat")
            nc.tensor.transpose(out=ps_cat[:], in_=o_cat[:, t, :],
                                identity=ident[:])
            catT = work.tile([P, P], f16, tag="catT")
            nc.scalar.copy(catT[:], ps_cat[:])
            # gate^T = sigmoid(z + bl) = 1 / (1 + exp(-(z + bl)))  [C, P]
            ps_z = eps.tile([NCLASS, P], f32, tag="z", bufs=1)
            nc.tensor.matmul(ps_z[:], lhsT=wl_t[:], rhs=catT[:],
                             start=True, stop=True)
            eneg = work.tile([NCLASS, P], f16, tag="eneg")
            nc.scalar.activation(out=eneg[:], in_=ps_z[:], func=ACT.Exp,
                                 bias=nbl_t[:], scale=-1.0)
            gt = work.tile([NCLASS, P], f16, tag="gt")
            nc.vector.tensor_scalar(out=gt[:], in0=eneg[:], scalar1=1.0,
                                    scalar2=None, op0=ALU.add)
            with nc.allow_low_precision(reason="gate in (0,1); f16 ample"):
                nc.vector.reciprocal(gt[:], gt[:])
            # gate [P, C] via PE transpose
            ps_g = eps.tile([P, NCLASS], f16, tag="g", bufs=1)
            nc.tensor.transpose(out=ps_g[:], in_=gt[:],
                                identity=ident[0:NCLASS, 0:NCLASS])
            # out = o2 + g * (o1 - o2)
            dif = work.tile([P, NCLASS], f16, tag="dif")
            nc.vector.tensor_tensor(out=dif[:], in0=o1, in1=o2,
                                    op=ALU.subtract)
            nc.vector.tensor_tensor(out=dif[:], in0=ps_g[:], in1=dif[:],
                                    op=ALU.mult)
            nc.vector.tensor_tensor(out=l_all[:, t, :], in0=o2, in1=dif[:],
                                    op=ALU.add)
            nc.vector.tensor_reduce(
                out=negmax_all[:, t:t + 1], in_=l_all[:, t, :],
                axis=mybir.AxisListType.X, op=ALU.max, negate=True,
            )
            etmp = work.tile([P, NCLASS], f16, tag="etmp")
            nc.scalar.activation(
                out=etmp[:], in_=l_all[:, t, :], func=ACT.Exp,
                bias=negmax_all[:, t:t + 1], scale=1.0,
                accum_out=esum_all[:, t:t + 1],
            )
            # stream the tail per 7-tile group: one Ln (bounded act-table
            # switches), finals, and the output DMA
            if t % 7 == 6 or t == NTILE - 1:
                lo = t - t % 7
                nc.scalar.activation(out=lse_all[:, lo:t + 1],
                                     in_=esum_all[:, lo:t + 1], func=ACT.Ln)
                for u in range(lo, t + 1):
                    nc.vector.tensor_scalar(
                        out=ob[:, u, :], in0=l_all[:, u, :],
                        scalar1=negmax_all[:, u:u + 1],
                        scalar2=lse_all[:, u:u + 1],
                        op0=ALU.add, op1=ALU.subtract,
                    )
                nc.sync.dma_start(orr[:, lo:t + 1, :], ob[:, lo:t + 1, :])

        for tw in (0, 1):
            plan = plans[tw]
            ps_tiles = {}
            for c in range(plan.nchunks):
                b, g = divmod(c, G)
                msgs, sel = _emit_spmm_batches(nc, state, tw, b)
                for t, w, is_f, is_l in plan.cevents[c]:
                    if is_f:
                        ps_tiles[t] = acc_pool.tile([P, NCLASS], f32,
                                                    tag="acc",
                                                    name=f"acc{tw}_{t}")
                    nc.tensor.matmul(
                        ps_tiles[t][:], lhsT=sel[:, g, w * P:(w + 1) * P],
                        rhs=msgs[:, g, 0:NCLASS],
                        start=is_f, stop=False,
                    )
                    if not is_l:
                        continue
                    # bias add: rank-1 matmul ones^T @ b_row, carries stop
                    ps_o = ps_tiles.pop(t)
                    nc.tensor.matmul(
                        ps_o[:], lhsT=ones_t[:],
                        rhs=b24_t[:, tw * NCLASS:(tw + 1) * NCLASS],
                        start=False, stop=True,
                    )
                    nc.scalar.copy(o_cat[:, t, tw * 64:tw * 64 + NCLASS],
                                   ps_o[:])
                    if tw == 1:
                        fuse_tile(t)
    nc.compile()
    return nc


# ---------------------------------------------------------------- driver

TRACE = False          # set by test.py to collect per-launch artifacts
LAST_NCS = []          # built Bass modules per launch when TRACE


def _run(nc, in_maps):
    if TRACE:
        LAST_NCS.append(nc)
    return run_bass_kernel_spmd(nc, in_maps, core_ids=list(range(NCORES)))


def _make_nc():
    return bacc.Bacc(
        "TRN2", target_bir_lowering=False, debug=False,
        num_devices=NCORES, num_swdge_queues=2,
    )


def kernel(x, edge_index, edge_vals, edge_index2, edge_vals2,
           W1, b1, W2, b2, W3, b3, W4, b4, Wl, bl):
    x = np.asarray(x, np.float32)
    plans = [TowerPlan(edge_index, edge_vals), TowerPlan(edge_index2, edge_vals2)]

    def edge_inmap(c):
        m = {}
        for tw in (0, 1):
            m[f"idx{tw}"] = plans[tw].idx[c]
            m[f"dl{tw}"] = plans[tw].dl[c]
            m[f"vl{tw}"] = plans[tw].vl[c]
        return m

    # ---- launch A: xW = x @ [W1|W3]
    w13 = np.concatenate([np.asarray(W1, np.float32),
                          np.asarray(W3, np.float32)], axis=1)
    nc = _make_nc()
    build_A(nc)
    in_maps = []
    for c in range(NCORES):
        xT = np.zeros((NFEAT, NLOCP), np.float32)
        xT[:, :NLOC] = x[c * NLOC:(c + 1) * NLOC].T
        in_maps.append({"xT": xT, "w13": w13})
    res = _run(nc, in_maps)
    xw = np.zeros((NPAD, 2 * NHID), NP_F8 if XW_F8 else np.float16)
    for c in range(NCORES):
        xw[c * NLOC:(c + 1) * NLOC] = res.results[c]["out"][:NLOC]

    # ---- launch B: h = relu(spmm(xW) + b); hW2
    w24 = np.concatenate([np.asarray(W2, np.float32),
                          np.asarray(W4, np.float32)], axis=1).astype(np.float16)
    b13 = np.stack([np.asarray(b1, np.float32),
                    np.asarray(b3, np.float32)], axis=1)
    nc = _make_nc()
    build_B(nc, plans)
    in_maps = [{"xw": xw, "w24": w24, "b13": b13, **edge_inmap(c)}
               for c in range(NCORES)]
    res = _run(nc, in_maps)
    hw2 = np.zeros((NPAD, 128), np.float16)
    for c in range(NCORES):
        o = res.results[c]["out"][:NLOC]
        hw2[c * NLOC:(c + 1) * NLOC, 0:NCLASS] = o[:, 0:NCLASS]
        hw2[c * NLOC:(c + 1) * NLOC, 64:64 + NCLASS] = o[:, NCLASS:2 * NCLASS]

    # ---- launch C: o = spmm(hW2) + b; gated fusion; log_softmax
    wl_f = np.asarray(Wl, np.float32).astype(np.float16)      # [2C, C]
    wl_h = np.zeros((P, NCLASS), np.float16)
    wl_h[0:NCLASS] = wl_f[0:NCLASS]
    wl_h[64:64 + NCLASS] = wl_f[NCLASS:2 * NCLASS]
    b24 = np.stack([np.asarray(b2, np.float32),
                    np.asarray(b4, np.float32)]).reshape(1, 2 * NCLASS)
    b24 = b24.astype(np.float16)
    nbl_c = -np.asarray(bl, np.float32).reshape(NCLASS, 1)
    nc = _make_nc()
    build_C(nc, plans)
    in_maps = [{"hw2": hw2, "wl": wl_h, "b24": b24, "nbl": nbl_c, **edge_inmap(c)}
               for c in range(NCORES)]
    res = _run(nc, in_maps)
    out = np.zeros((N, NCLASS), np.float32)
    for c in range(NCORES):
        out[c * NLOC:(c + 1) * NLOC] = res.results[c]["out"][:NLOC]
    return out


# revision 42
# speedup vs baseline: 1.0407x; 1.0407x over previous
"""Trainium2 Bass kernel for the two-tower GCN (nn_GCN2).

Distribution: nodes partitioned by destination range across 8 cores
(graph parallel). Edge lists are preprocessed on host (index manipulation
only): assigned to the core owning their dst node, sorted by dst tile,
and padded so every core runs the identical program. Gather indices are
stored relative to the table midpoint (signed int16 sign-extends in the
gather ucode), so one index stream covers all 50176 table rows.

All floating-point math runs on device across 3 SPMD launches:
  A: xW   = x @ [W1|W3]  (f16 matmul)
  B: h^T  = relu(spmm(A, xW) + b), o = hW2      (per dst tile, fp8 gather)
  C: oT   = spmm(A, hW2); gated fusion; log_softmax  ([dst, class] layout)

The irregular segment-sum runs as selector-matrix matmuls on the tensor
engine: for each chunk of 128 edges, sel[e, d] = val_e * (dstloc_e == d)
is built on the vector engine from a constant iota, and the per-edge
gathered feature rows (via dma_gather) contract against it in PSUM.
"""
from contextlib import ExitStack

import numpy as np

import concourse.bass as bass
import concourse.tile as tile
from concourse import bacc, mybir
from concourse.bass_utils import run_bass_kernel_spmd
from concourse.masks import make_identity

P = 128
NCORES = 8
N = 50000
NFEAT = 512
NHID = 128
NCLASS = 40
NLOC = N // NCORES            # 6250 real nodes per core
NTILE = (NLOC + P - 1) // P   # 49 dst tiles per core
NLOCP = NTILE * P             # 6272 padded rows per core
NPAD = NCORES * NLOCP         # 50176 padded table rows
MID = NPAD // 2               # gather base row (signed idx spans the table)
G = 8                         # chunks per gather batch
R = G * P                     # 1024 indices per dma_gather (ucode max)

f8 = mybir.dt.float8e4
f16 = mybir.dt.float16
f32 = mybir.dt.float32
i16 = mybir.dt.int16
i32 = mybir.dt.int32
ACT = mybir.ActivationFunctionType
ALU = mybir.AluOpType

NP_F8 = mybir.dt.np(f8)
XW_F8 = False                 # launch-B gather table in fp8 (else f16)


def _cdiv(a, b):
    return (a + b - 1) // b


# ---------------------------------------------------------------- host prep

class TowerPlan:
    """Edge preprocessing for one tower (one graph).

    Chunks are plain 128-edge groups of the per-core dst-sorted stream (no
    per-tile padding). A uniform chunk->tile-window map (win/wide/cevents)
    lets every core run the same program: boundary chunks emit one matmul
    per covered tile with a 256-wide selector; zero selector columns make
    over-approximation harmless.

    idx        : [NCORES, nb, 128, R//16] int16 mid-relative gather indices
    dl, vl     : [NCORES, 128, nchunks] window-relative dstloc / edge values
    cevents[c] : [(tile, window_offset, is_first, is_last)]
    """

    def __init__(self, edge_index, edge_vals):
        src = np.asarray(edge_index[0]).astype(np.int64)
        dst = np.asarray(edge_index[1]).astype(np.int64)
        vals = np.asarray(edge_vals).astype(np.float32)

        core = dst // NLOC
        ldst = dst - core * NLOC

        counts = np.bincount(core, minlength=NCORES)              # [NCORES]
        self.nchunks = max(int(_cdiv(counts.max(), P)), 1)
        self.nb = _cdiv(self.nchunks, G)
        self.last_R = (self.nchunks - (self.nb - 1) * G) * P

        # slot position of each edge: rank within the core's dst-sorted
        # stream — chunks are NOT tile-aligned (no per-tile padding)
        order = np.lexsort((ldst, core))
        so_core = core[order]
        so_ldst, so_src, so_val = ldst[order], src[order], vals[order]
        gstart = np.r_[0, np.flatnonzero(np.diff(so_core)) + 1]
        glen = np.diff(np.r_[gstart, len(so_core)])
        rank = np.arange(len(so_core)) - np.repeat(gstart, glen)

        nslot = self.nchunks * P
        nb = self.nb
        srcrel = np.zeros((NCORES, nslot), np.int32)   # pad idx 0 (row MID)
        dla = np.zeros((NCORES, nslot), np.float32)
        vla = np.zeros((NCORES, nslot), np.float32)    # pad val 0
        lda = np.full((NCORES, nslot), -1, np.int64)   # ldst, -1 = pad
        flat = so_core * nslot + rank
        srcrel.reshape(-1)[flat] = (so_src - MID).astype(np.int32)
        lda.reshape(-1)[flat] = so_ldst
        vla.reshape(-1)[flat] = so_val

        # uniform chunk -> tile-window map: W[c] = min over cores of the
        # first real edge's tile; chunks span <= 2 consecutive tiles across
        # all cores (inter-core quantile jitter << 128 dsts)
        ldc = lda.reshape(NCORES, self.nchunks, P)
        real = ldc >= 0
        ftile = np.where(real, ldc // P, NTILE).min(axis=2)       # [NCORES, nc]
        ltile = np.where(real, ldc // P, -1).max(axis=2)
        W = ftile.min(axis=0)
        mx = ltile.max(axis=0)
        # pad-only chunks (tail): pin to the last window
        none = mx < 0
        W[none] = NTILE - 1
        mx[none] = NTILE - 1
        W = np.minimum(W, mx)
        assert (mx - W <= 1).all(), "chunk spans >2 tiles"
        self.win = W                                   # [nchunks]
        self.wide = mx - W                             # 0 = single tile
        dla.reshape(NCORES, self.nchunks, P)[:] = np.where(
            real, ldc - (W * P)[None, :, None], 0.0)

        # per chunk: (tile, window-offset, is_first, is_last) events
        first_c = np.full(NTILE, -1)
        last_c = np.zeros(NTILE, np.int64)
        for c in range(self.nchunks):
            for t in range(W[c], mx[c] + 1):
                if first_c[t] < 0:
                    first_c[t] = c
                last_c[t] = c
        self.cevents = [[] for _ in range(self.nchunks)]
        for c in range(self.nchunks):
            for t in range(W[c], mx[c] + 1):
                self.cevents[c].append(
                    (t, t - W[c], first_c[t] == c, last_c[t] == c))

        # The gather ucode stops at the last non-negative index (trailing
        # negatives read as padding). Swap a non-negative slot into each
        # batch's final position; slot order within a chunk is free.
        for cc in range(NCORES):
            for b in range(nb):
                last = min((b + 1) * R, nslot) - 1
                if srcrel[cc, last] >= 0:
                    continue
                c0 = (last // P) * P
                j = c0 + int(np.argmax(srcrel[cc, c0:last + 1] >= 0))
                assert srcrel[cc, j] >= 0, "all-negative chunk tail"
                for arr in (srcrel, dla, vla):
                    arr[cc, [j, last]] = arr[cc, [last, j]]

        # wrapped idx [NCORES, nb, 128, R//16]; pad idx 0 beyond nslot
        w = np.zeros((NCORES, nb * R), np.int32)
        w[:, :nslot] = srcrel
        w = w.reshape(NCORES, nb, R)
        jj = np.arange(R)
        wr = np.zeros((NCORES, nb, 16, R // 16), np.int16)
        wr[:, :, jj % 16, jj // 16] = w.astype(np.int16)
        self.idx = np.tile(wr, (1, 1, 8, 1))           # [NCORES, nb, 128, R//16]

        assert (first_c >= 0).all()

        def colmaj(a):
            out = a.reshape(NCORES, self.nchunks, P).astype(np.float32)
            return np.ascontiguousarray(out.transpose(0, 2, 1))

        self.dl = colmaj(dla)
        self.vl = colmaj(vla)


# ---------------------------------------------------------------- kernels

def _dma_gather_small(gp, out_ap, in_ap, idxs_ap, num_idxs, num_idxs_reg,
                      elem_size, elem_step, queue_num=0):
    """dma_gather for elem sizes below 256B (non-transpose DRAM path only).

    bass.dma_gather asserts elem_size_bytes % 256 == 0, but that alignment is
    only required by the transpose ucode; the plain path only needs the row
    stride in 256B units. Mirrors bass.py's lowering minus that assert.
    """
    from concourse import ap_utils
    from concourse._compat import exact_div
    assert idxs_ap.dtype == i16
    assert in_ap.dtype == out_ap.dtype
    assert in_ap.space == bass.MemorySpace.DRAM
    assert ap_utils.ap_is_contiguous(out_ap.ap[1:])
    assert ap_utils.ap_is_contiguous(idxs_ap.ap[1:])
    assert in_ap.ap[-1][1] == out_ap.ap[-1][1] == elem_size
    assert out_ap.ap[0][1] * out_ap.ap[1][1] == _cdiv(num_idxs, P) * P
    assert in_ap.ap[0][0] == elem_step
    stride_bytes = elem_step * mybir.dt.size(in_ap.dtype)
    stride_bytes_256 = exact_div(stride_bytes, 256)
    _in_ap = gp.lower_ap_dma(in_ap, for_custom_bir_dma=True)
    _idxs_ap = gp.lower_ap(idxs_ap)
    _out_ap = gp.lower_ap(out_ap)
    return gp.add_instruction(mybir.InstDMAGatherAnt(
        name=gp.bass.get_next_instruction_name(),
        ins=[*_in_ap, _idxs_ap, gp.lower_val_access(gp.to_reg(num_idxs_reg))],
        outs=[_out_ap],
        transpose=False, num_idxs=num_idxs, elem_size=elem_size,
        stride_bytes_256=stride_bytes_256, gen_mode=0, single_packet=True,
        queue_num=queue_num, sbuf_tokens_per_rank=0, sbuf_free_dim_per_rank=0,
        sbuf_free_dim_pad_per_rank=0, sbuf_byte_offset=0,
    ))


def _iota_const(nc, ctx, tc):
    pool = ctx.enter_context(tc.tile_pool(name="iotac", bufs=1))
    it32 = pool.tile([P, 2 * P], i32)
    nc.gpsimd.iota(it32[:], pattern=[[1, 2 * P]], base=0, channel_multiplier=0)
    it16 = pool.tile([P, 2 * P], f16)
    nc.vector.tensor_copy(it16[:], it32[:])
    return it16


def build_A(nc):
    xT = nc.dram_tensor("xT", [NFEAT, NLOCP], f32, kind="ExternalInput").ap()
    w13 = nc.dram_tensor("w13", [NFEAT, 2 * NHID], f32, kind="ExternalInput").ap()
    odt = f8 if XW_F8 else f16
    out = nc.dram_tensor("out", [NLOCP, 2 * NHID], odt, kind="ExternalOutput").ap()
    KCH = NFEAT // P  # 4

    TB = 7                    # dst tiles per column block
    NBLK = NTILE // TB        # 7 blocks
    COLB = TB * P             # 896

    with tile.TileContext(nc) as tc, ExitStack() as ctx:
        big = ctx.enter_context(tc.tile_pool(name="big", bufs=1))
        xf_pool = ctx.enter_context(tc.tile_pool(name="xf", bufs=5))
        psum = ctx.enter_context(tc.tile_pool(name="ps", bufs=4, space="PSUM"))

        w_t = []
        for k in range(KCH):
            t = big.tile([P, 2 * NHID], f32, tag=f"w{k}")
            nc.sync.dma_start(t[:], w13[k * P:(k + 1) * P, :])
            tb = big.tile([P, 2 * NHID], f16, tag=f"wb{k}")
            nc.vector.tensor_copy(tb[:], t[:])
            w_t.append(tb)
        ob = big.tile([P, NTILE, 2 * NHID], odt, tag="ob")

        orr = out.rearrange("(t p) f -> p t f", p=P)
        for blk in range(NBLK):
            xt_t = []
            for k in range(KCH):
                t = xf_pool.tile([P, COLB], f32, tag=f"xt{k}")
                nc.sync.dma_start(
                    t[:], xT[k * P:(k + 1) * P, blk * COLB:(blk + 1) * COLB]
                )
                tb = xf_pool.tile([P, COLB], f16, tag=f"xb{k}")
                nc.scalar.copy(tb[:], t[:])
                xt_t.append(tb)
            for rr in range(TB):
                r = blk * TB + rr
                ps = psum.tile([P, 2 * NHID], f32, tag="ps")
                for k in range(KCH):
                    nc.tensor.matmul(
                        ps[:],
                        lhsT=xt_t[k][:, rr * P:(rr + 1) * P],
                        rhs=w_t[k][:],
                        start=(k == 0), stop=(k == KCH - 1),
                    )
                nc.vector.tensor_copy(ob[:, r, :], ps[:])
            nc.sync.dma_start(orr[:, blk * TB:(blk + 1) * TB, :],
                              ob[:, blk * TB:(blk + 1) * TB, :])
    nc.compile()
    return nc


def _emit_spmm_batches(nc, state, tw, b):
    """Lazily emit gather + selector build for batch b of tower tw."""
    key = (tw, b)
    if key in state["batches"]:
        return state["batches"][key]
    plan, pools = state["plans"][tw], state["pools"]
    iota = state["iota"]
    nbq = state["q"]
    state["q"] += 1

    elem = state["elem"]
    nidx = plan.last_R if b == plan.nb - 1 else R
    gcnt = nidx // P
    msgs = pools["msgs"].tile([P, G, elem], state["mdt"], tag="msgs")
    _dma_gather_small(
        nc.gpsimd, msgs[:, 0:gcnt, :], state["tabs"][tw],
        state["idx"][tw][:, b, 0:nidx // 16],
        num_idxs=nidx, num_idxs_reg=nidx,
        elem_size=elem, elem_step=state["tab_step"],
        queue_num=nbq % 2,
    )
    sel = pools["sel"].tile([P, G, 2 * P], f16, tag="sel")
    dl = state["dl"][tw]
    vl = state["vl"][tw]
    for g in range(gcnt):
        c = b * G + g
        wd = (1 + plan.wide[c]) * P
        nc.vector.tensor_scalar(
            out=sel[:, g, 0:wd], in0=iota[:, 0:wd],
            scalar1=dl[:, c:c + 1], scalar2=vl[:, c:c + 1],
            op0=ALU.is_equal, op1=ALU.mult,
        )
    state["batches"][key] = (msgs, sel)
    return msgs, sel


def _load_edge_inputs(nc, ctx, tc, plans):
    """Declare + load idx/dl/vl tensors for both towers.

    Loads are split so the first gathers and selector builds only wait on
    a small head slice, not the whole multi-MB edge stream.
    """
    idx_t, dl_t, vl_t = {}, {}, {}
    pool = ctx.enter_context(tc.tile_pool(name="edges", bufs=1))
    HB = 10                    # head: 10 gather batches worth of everything
    SB = 512                   # tail slice: one DMA per array
    tails = []                 # (position_key, dst_ap, src_ap)
    for tw in (0, 1):
        plan = plans[tw]
        nb, nch = plan.nb, plan.nchunks
        d_idx = nc.dram_tensor(
            f"idx{tw}", [nb, P, R // 16], i16, kind="ExternalInput"
        ).ap()
        t_idx = pool.tile([P, nb, R // 16], i16, tag=f"idx{tw}")
        d_idx_r = d_idx.rearrange("b p w -> p b w")
        nc.sync.dma_start(t_idx[:, 0:HB, :], d_idx_r[:, 0:HB, :])
        idx_t[tw] = t_idx
        d_dl = nc.dram_tensor(
            f"dl{tw}", [P, nch], f32, kind="ExternalInput"
        ).ap()
        t_dl = pool.tile([P, nch], f32, tag=f"dl{tw}")
        nc.sync.dma_start(t_dl[:, 0:HB * G], d_dl[:, 0:HB * G])
        dl_t[tw] = t_dl
        d_vl = nc.dram_tensor(
            f"vl{tw}", [P, nch], f32, kind="ExternalInput"
        ).ap()
        t_vl = pool.tile([P, nch], f32, tag=f"vl{tw}")
        nc.sync.dma_start(t_vl[:, 0:HB * G], d_vl[:, 0:HB * G])
        vl_t[tw] = t_vl
        for b0 in range(HB, nb, SB):
            b1 = min(b0 + SB, nb)
            c0, c1 = b0 * G, min(b1 * G, nch)
            tails.append((b0, t_idx[:, b0:b1, :], d_idx_r[:, b0:b1, :]))
            tails.append((b0, t_dl[:, c0:c1], d_dl[:, c0:c1]))
            tails.append((b0, t_vl[:, c0:c1], d_vl[:, c0:c1]))
    for _, dst, src in sorted(tails, key=lambda x: x[0]):
        nc.sync.dma_start(dst, src)
    return idx_t, dl_t, vl_t


def build_B(nc, plans):
    tdt = f8 if XW_F8 else f16
    xw = nc.dram_tensor("xw", [NPAD, 2 * NHID], tdt, kind="ExternalInput").ap()
    w24 = nc.dram_tensor("w24", [NHID, 2 * NCLASS], f16, kind="ExternalInput").ap()
    b13 = nc.dram_tensor("b13", [NHID, 2], f32, kind="ExternalInput").ap()
    out = nc.dram_tensor("out", [NLOCP, 2 * NCLASS], f16, kind="ExternalOutput").ap()

    with tile.TileContext(nc) as tc, ExitStack() as ctx:
        iota = _iota_const(nc, ctx, tc)
        idx_t, dl_t, vl_t = _load_edge_inputs(nc, ctx, tc, plans)
        consts = ctx.enter_context(tc.tile_pool(name="consts", bufs=1))
        w24_t = consts.tile([NHID, 2 * NCLASS], f16)
        nc.sync.dma_start(w24_t[:], w24[:])
        b13_t = consts.tile([NHID, 2], f32)
        nc.sync.dma_start(b13_t[:], b13[:])
        ob = consts.tile([P, NTILE, 2 * NCLASS], f16, tag="ob")

        pools = {
            "msgs": ctx.enter_context(tc.tile_pool(name="msgs", bufs=12)),
            "sel": ctx.enter_context(tc.tile_pool(name="sel", bufs=12)),
        }
        psum = ctx.enter_context(tc.tile_pool(name="ps", bufs=3, space="PSUM"))
        psum_o = ctx.enter_context(tc.tile_pool(name="pso", bufs=2, space="PSUM"))
        hpool = ctx.enter_context(tc.tile_pool(name="h", bufs=3))

        state = {
            "plans": plans, "pools": pools, "iota": iota, "q": 0,
            "batches": {}, "idx": idx_t, "dl": dl_t, "vl": vl_t,
            "tabs": [xw[MID:, 0:NHID], xw[MID:, NHID:2 * NHID]],
            "tab_step": 2 * NHID,
            "elem": NHID,
            "mdt": tdt,
        }

        orr = out.rearrange("(t p) f -> p t f", p=P)
        for tw in (0, 1):
            plan = plans[tw]
            ps_tiles = {}
            for c in range(plan.nchunks):
                b, g = divmod(c, G)
                msgs, sel = _emit_spmm_batches(nc, state, tw, b)
                for t, w, is_f, is_l in plan.cevents[c]:
                    if is_f:
                        ps_tiles[t] = psum.tile([NHID, P], f32, tag="psh",
                                                name=f"psh{tw}_{t}")
                    nc.tensor.matmul(
                        ps_tiles[t][:], lhsT=msgs[:, g, :],
                        rhs=sel[:, g, w * P:(w + 1) * P],
                        start=is_f, stop=is_l,
                    )
                    if not is_l:
                        continue
                    ps_h = ps_tiles.pop(t)
                    hT = hpool.tile([NHID, P], f16, tag="hT")
                    nc.scalar.activation(
                        out=hT[:], in_=ps_h[:], func=ACT.Relu,
                        bias=b13_t[:, tw:tw + 1], scale=1.0,
                    )
                    ps_o = psum_o.tile([P, NCLASS], f32, tag="pso")
                    nc.tensor.matmul(
                        ps_o[:], lhsT=hT[:],
                        rhs=w24_t[:, tw * NCLASS:(tw + 1) * NCLASS],
                        start=True, stop=True,
                    )
                    nc.scalar.copy(ob[:, t, tw * NCLASS:(tw + 1) * NCLASS],
                                   ps_o[:])
                    # stream output per tile group once both towers are done
                    if tw == 1 and (t % 7 == 6 or t == NTILE - 1):
                        lo = t - t % 7
                        nc.sync.dma_start(orr[:, lo:t + 1, :],
                                          ob[:, lo:t + 1, :])
    nc.compile()
    return nc


def build_C(nc, plans):
    # hw2 table: f16, 256B row stride; tower tw's 40 cols start at tw*64
    hw2 = nc.dram_tensor("hw2", [NPAD, 128], f16, kind="ExternalInput").ap()
    # wl padded to 128 rows: rows 0:40 = Wl[0:40], rows 64:104 = Wl[40:80]
    wl = nc.dram_tensor("wl", [P, NCLASS], f16, kind="ExternalInput").ap()
    b24 = nc.dram_tensor("b24", [1, 2 * NCLASS], f16, kind="ExternalInput").ap()
    nbl = nc.dram_tensor("nbl", [NCLASS, 1], f32, kind="ExternalInput").ap()
    out = nc.dram_tensor("out", [NLOCP, NCLASS], f32, kind="ExternalOutput").ap()

    with tile.TileContext(nc) as tc, ExitStack() as ctx:
        iota = _iota_const(nc, ctx, tc)
        idx_t, dl_t, vl_t = _load_edge_inputs(nc, ctx, tc, plans)
        consts = ctx.enter_context(tc.tile_pool(name="consts", bufs=1))
        wl_t = consts.tile([P, NCLASS], f16)
        nc.sync.dma_start(wl_t[:], wl[:])
        b24_t = consts.tile([1, 2 * NCLASS], f16)
        nc.sync.dma_start(b24_t[:], b24[:])
        nbl_t = consts.tile([NCLASS, 1], f32)
        nc.sync.dma_start(nbl_t[:], nbl[:])
        ones_t = consts.tile([1, P], f16, tag="ones")
        nc.vector.memset(ones_t[:], 1.0)
        ident = consts.tile([P, P], f16, tag="ident")
        make_identity(nc, ident[:])
        # fused per-tower outputs [dst, (tw0 cols 0:40 | tw1 cols 64:104)];
        # zeroed once so the cat transpose emits clean zero fill rows
        o_cat = consts.tile([P, NTILE, P], f16, tag="o_cat")
        nc.vector.memset(o_cat[:], 0.0)
        # logits + softmax stats, ln'd once at the end
        l_all = consts.tile([P, NTILE, NCLASS], f16, tag="l_all")
        ob = consts.tile([P, NTILE, NCLASS], f32, tag="ob")
        negmax_all = consts.tile([P, NTILE], f32, tag="negmax")
        esum_all = consts.tile([P, NTILE], f32, tag="esum")
        lse_all = consts.tile([P, NTILE], f32, tag="lse")

        pools = {
            "msgs": ctx.enter_context(tc.tile_pool(name="msgs", bufs=12)),
            "sel": ctx.enter_context(tc.tile_pool(name="sel", bufs=12)),
        }
        work = ctx.enter_context(tc.tile_pool(name="work", bufs=4))

        state = {
            "plans": plans, "pools": pools, "iota": iota, "q": 0,
            "batches": {}, "idx": idx_t, "dl": dl_t, "vl": vl_t,
            "tabs": [hw2[MID:, 0:64], hw2[MID:, 64:128]],
            "tab_step": 128,
            "elem": 64,
            "mdt": f16,
        }

        # ---- per tile: accumulate both towers' spmm in PSUM, then fuse
        acc_pool = ctx.enter_context(tc.tile_pool(name="acc", bufs=4,
                                                  space="PSUM"))
        eps = ctx.enter_context(tc.tile_pool(name="eps", bufs=2, space="PSUM"))
        orr = out.rearrange("(t p) f -> p t f", p=P)
        def fuse_tile(t):
            # gated fusion + log_softmax stats for tile t. Only Exp runs on
            # the scalar engine here -- any other activation function would
            # trigger a 1.3us table reload per switch.
            o1 = o_cat[:, t, 0:NCLASS]
            o2 = o_cat[:, t, 64:64 + NCLASS]
            # catT [128, P]: one transpose; fill rows come out zero
            ps_cat = eps.tile([P, P], f16, tag="cat")
            nc.tensor.transpose(out=ps_cat[:], in_=o_cat[:, t, :],
                                identity=ident[:])
            catT = work.tile([P, P], f16, tag="catT")
            nc.scalar.copy(catT[:], ps_cat[:])
            # gate^T = sigmoid(z + bl) = 1 / (1 + exp(-(z + bl)))  [C, P]
            ps_z = eps.tile([NCLASS, P], f32, tag="z", bufs=1)
            nc.tensor.matmul(ps_z[:], lhsT=wl_t[:], rhs=catT[:],
                             start=True, stop=True)
            eneg = work.tile([NCLASS, P], f16, tag="eneg")
            nc.scalar.activation(out=eneg[:], in_=ps_z[:], func=ACT.Exp,
                                 bias=nbl_t[:], scale=-1.0)
            gt = work.tile([NCLASS, P], f16, tag="gt")
            nc.vector.tensor_scalar(out=gt[:], in0=eneg[:], scalar1=1.0,
                                    scalar2=None, op0=ALU.add)
            with nc.allow_low_precision(reason="gate in (0,1); f16 ample"):
                nc.vector.reciprocal(gt[:], gt[:])
            # gate [P, C] via PE transpose
            ps_g = eps.tile([P, NCLASS], f16, tag="g", bufs=1)
            nc.tensor.transpose(out=ps_g[:], in_=gt[:],
                                identity=ident[0:NCLASS, 0:NCLASS])
            # out = o2 + g * (o1 - o2)
            dif = work.tile([P, NCLASS], f16, tag="dif")
            nc.vector.tensor_tensor(out=dif[:], in0=o1, in1=o2,
                                    op=ALU.subtract)
            nc.vector.tensor_tensor(out=dif[:], in0=ps_g[:], in1=dif[:],
                                    op=ALU.mult)
            nc.vector.tensor_tensor(out=l_all[:, t, :], in0=o2, in1=dif[:],
                                    op=ALU.add)
            nc.vector.tensor_reduce(
                out=negmax_all[:, t:t + 1], in_=l_all[:, t, :],
                axis=mybir.AxisListType.X, op=ALU.max, negate=True,
            )
            etmp = work.tile([P, NCLASS], f16, tag="etmp")
            nc.scalar.activation(
                out=etmp[:], in_=l_all[:, t, :], func=ACT.Exp,
                bias=negmax_all[:, t:t + 1], scale=1.0,
                accum_out=esum_all[:, t:t + 1],
            )

        # Interleave the towers' chunk streams at gather-batch granularity so
        # per-tile fusion overlaps accumulation throughout the launch. Tower
        # completion positions jitter by a few chunks, so fusion fires only
        # once BOTH towers' evictions for a tile are emitted, and output
        # groups flush by completion count, not tile order.
        done = [set(), set()]
        gleft = [min(7, NTILE - lo) for lo in range(0, NTILE, 7)]

        def finish_tile(t):
            fuse_tile(t)
            gi = t // 7
            gleft[gi] -= 1
            if gleft[gi] == 0:
                lo = gi * 7
                hi = min(lo + 7, NTILE)
                nc.scalar.activation(out=lse_all[:, lo:hi],
                                     in_=esum_all[:, lo:hi], func=ACT.Ln)
                for u in range(lo, hi):
                    nc.vector.tensor_scalar(
                        out=ob[:, u, :], in0=l_all[:, u, :],
                        scalar1=negmax_all[:, u:u + 1],
                        scalar2=lse_all[:, u:u + 1],
                        op0=ALU.add, op1=ALU.subtract,
                    )
                nc.sync.dma_start(orr[:, lo:hi, :], ob[:, lo:hi, :])

        ps_tiles = {}
        for b in range(max(plans[0].nb, plans[1].nb)):
            for tw in (0, 1):
                plan = plans[tw]
                if b >= plan.nb:
                    continue
                nidx = plan.last_R if b == plan.nb - 1 else R
                msgs, sel = _emit_spmm_batches(nc, state, tw, b)
                for g in range(nidx // P):
                    c = b * G + g
                    for t, w, is_f, is_l in plan.cevents[c]:
                        if is_f:
                            ps_tiles[(tw, t)] = acc_pool.tile(
                                [P, NCLASS], f32, tag="acc",
                                name=f"acc{tw}_{t}")
                        nc.tensor.matmul(
                            ps_tiles[(tw, t)][:],
                            lhsT=sel[:, g, w * P:(w + 1) * P],
                            rhs=msgs[:, g, 0:NCLASS],
                            start=is_f, stop=False,
                        )
                        if not is_l:
                            continue
                        # bias add: rank-1 matmul, carries the stop flag
                        ps_o = ps_tiles.pop((tw, t))
                        nc.tensor.matmul(
                            ps_o[:], lhsT=ones_t[:],
                            rhs=b24_t[:, tw * NCLASS:(tw + 1) * NCLASS],
                            start=False, stop=True,
                        )
                        nc.scalar.copy(
                            o_cat[:, t, tw * 64:tw * 64 + NCLASS], ps_o[:])
                        done[tw].add(t)
                        if t in done[1 - tw]:
                            finish_tile(t)
    nc.compile()
    return nc


# ---------------------------------------------------------------- driver

TRACE = False          # set by test.py to collect per-launch artifacts
LAST_NCS = []          # built Bass modules per launch when TRACE


def _run(nc, in_maps):
    if TRACE:
        LAST_NCS.append(nc)
    return run_bass_kernel_spmd(nc, in_maps, core_ids=list(range(NCORES)))


def _make_nc():
    return bacc.Bacc(
        "TRN2", target_bir_lowering=False, debug=False,
        num_devices=NCORES, num_swdge_queues=2,
    )


def kernel(x, edge_index, edge_vals, edge_index2, edge_vals2,
           W1, b1, W2, b2, W3, b3, W4, b4, Wl, bl):
    x = np.asarray(x, np.float32)
    plans = [TowerPlan(edge_index, edge_vals), TowerPlan(edge_index2, edge_vals2)]

    def edge_inmap(c):
        m = {}
        for tw in (0, 1):
            m[f"idx{tw}"] = plans[tw].idx[c]
            m[f"dl{tw}"] = plans[tw].dl[c]
            m[f"vl{tw}"] = plans[tw].vl[c]
        return m

    # ---- launch A: xW = x @ [W1|W3]
    w13 = np.concatenate([np.asarray(W1, np.float32),
                          np.asarray(W3, np.float32)], axis=1)
    nc = _make_nc()
    build_A(nc)
    in_maps = []
    for c in range(NCORES):
        xT = np.zeros((NFEAT, NLOCP), np.float32)
        xT[:, :NLOC] = x[c * NLOC:(c + 1) * NLOC].T
        in_maps.append({"xT": xT, "w13": w13})
    res = _run(nc, in_maps)
    xw = np.zeros((NPAD, 2 * NHID), NP_F8 if XW_F8 else np.float16)
    for c in range(NCORES):
        xw[c * NLOC:(c + 1) * NLOC] = res.results[c]["out"][:NLOC]

    # ---- launch B: h = relu(spmm(xW) + b); hW2
    w24 = np.concatenate([np.asarray(W2, np.float32),
                          np.asarray(W4, np.float32)], axis=1).astype(np.float16)
    b13 = np.stack([np.asarray(b1, np.float32),
                    np.asarray(b3, np.float32)], axis=1)
    nc = _make_nc()
    build_B(nc, plans)
    in_maps = [{"xw": xw, "w24": w24, "b13": b13, **edge_inmap(c)}
               for c in range(NCORES)]
    res = _run(nc, in_maps)
    hw2 = np.zeros((NPAD, 128), np.float16)
    for c in range(NCORES):
        o = res.results[c]["out"][:NLOC]
        hw2[c * NLOC:(c + 1) * NLOC, 0:NCLASS] = o[:, 0:NCLASS]
        hw2[c * NLOC:(c + 1) * NLOC, 64:64 + NCLASS] = o[:, NCLASS:2 * NCLASS]

    # ---- launch C: o = spmm(hW2) + b; gated fusion; log_softmax
    wl_f = np.asarray(Wl, np.float32).astype(np.float16)      # [2C, C]
    wl_h = np.zeros((P, NCLASS), np.float16)
    wl_h[0:NCLASS] = wl_f[0:NCLASS]
    wl_h[64:64 + NCLASS] = wl_f[NCLASS:2 * NCLASS]
    b24 = np.stack([np.asarray(b2, np.float32),
                    np.asarray(b4, np.float32)]).reshape(1, 2 * NCLASS)
    b24 = b24.astype(np.float16)
    nbl_c = -np.asarray(bl, np.float32).reshape(NCLASS, 1)
    nc = _make_nc()
    build_C(nc, plans)
    in_maps = [{"hw2": hw2, "wl": wl_h, "b24": b24, "nbl": nbl_c, **edge_inmap(c)}
               for c in range(NCORES)]
    res = _run(nc, in_maps)
    out = np.zeros((N, NCLASS), np.float32)
    for c in range(NCORES):
        out[c * NLOC:(c + 1) * NLOC] = res.results[c]["out"][:NLOC]
    return out
